# revision 12
# baseline (speedup 1.0000x reference)
"""DGCNN-style (2x DynamicEdgeConv + linear + global max pool) Trainium2 kernel.

Sharding: data-parallel over the batch dim — 8 NeuronCores x 4 graphs each.

The execution path charges a large fixed cost per STATIC instruction, so the
program is structured around For_i hardware loops to keep the static count
small (~150 user instructions vs ~5600 fully unrolled):

  for g in For_i(0, 4):                     # graphs
    kNN1: aug tensors, then for t in For_i(0,16): stage lhsT, 4 matmuls
          ([128,2048] distance row-tile in PSUM), exact top-24 via
          max8/max_index/match_replace directly on PSUM; one DMA stores
          [2048,20] uint16 neighbor ids.
    conv1: u' = x@Wu (per point, replicated halves), a' = x@Wa+c (2-half
          stack), one 20480-edge GPSIMD gather of u', broadcast-add a',
          DVE relu, then for c in For_i(0,8): 5 matmuls (blockdiag W2) +
          one segmented max-reduce (128 points per iter), + b2.
    kNN2: same scheme in 64-d feature space (contract dim 66).
    conv2: u = x1@W3b and c = x1@(W3a-W3b)+b3 per POINT (not per edge),
          gather u columns per edge (2 x 20480), segmented max, add c.
    lin1: [x1;1]@wl1_aug + x2@wl2 accumulated in PSUM, global max-reduce.
"""

import numpy as np

import concourse.mybir as mybir
from concourse import bacc
from concourse.bass import ds
from concourse.tile import TileContext
from concourse.bass_utils import run_bass_kernel_spmd

B, P, K = 32, 2048, 20
NCORES = 8
G = B // NCORES          # graphs per core
NT = P // 128            # 16 row-tiles per graph
EH = P * K // 2          # edges per point-half (20480)
NEG_BIG = -3.0e38

f32 = mybir.dt.float32
u16 = mybir.dt.uint16
i16 = mybir.dt.int16

_cache = {}

W_NAMES = dict(
    wa_blk=[8, 128], wu_rep=[4, 128], w2_blk=[128, 128], b2_2=[128, 1],
    w3b=[64, 128], w3c_aug=[65, 128], wl1_aug=[65, 128], wl2=[128, 128])


def _build_host_tensors(W1, b1, g1, bt1, W2, b2, W3, b3, Wl, bl):
    """Data-independent repackings of the weights (host-side, tiny)."""
    W1a, W1b = W1[:3], W1[3:]
    Wa = (W1a - W1b) * g1[None, :]          # [3,64]
    ca = b1 * g1 + bt1                      # [64]
    Wu = W1b * g1[None, :]                  # [3,64]

    # a'-matmul: out [128,1024] two point-halves stacked; lhsT blockdiag.
    wa_aug = np.concatenate([Wa, ca[None, :]], 0)      # [4,64]
    wa_blk = np.zeros((8, 128), np.float32)
    wa_blk[0:4, 0:64] = wa_aug
    wa_blk[4:8, 64:128] = wa_aug

    # u'-matmul: out [128,2048] = u' replicated on both partition halves.
    wu_aug = np.concatenate([Wu, np.zeros((1, 64), np.float32)], 0)  # [4,64]
    wu_rep = np.concatenate([wu_aug, wu_aug], 1)       # [4,128]

    w2_blk = np.zeros((128, 128), np.float32)
    w2_blk[0:64, 0:64] = W2
    w2_blk[64:128, 64:128] = W2
    b2_2 = np.concatenate([b2, b2])[:, None]           # [128,1]

    W3a, W3b = W3[:64], W3[64:]
    w3b = np.ascontiguousarray(W3b)                    # [64,128]
    w3c_aug = np.concatenate([W3a - W3b, b3[None, :]], 0)  # [65,128]

    wl1_aug = np.concatenate([Wl[:64], bl[None, :]], 0)    # [65,128]
    wl2 = np.ascontiguousarray(Wl[64:])                # [128,128]

    return dict(
        wa_blk=wa_blk, wu_rep=wu_rep, w2_blk=w2_blk, b2_2=b2_2,
        w3b=w3b, w3c_aug=w3c_aug, wl1_aug=wl1_aug, wl2=wl2)


def _emit_topk(nc, a_ps, m8, idxb, t):
    """Exact top-24 (desc) of each PSUM row -> idxb[:, t*24 : t*24+24] u16."""
    nc.vector.max(out=m8[:, 0:8], in_=a_ps)
    nc.vector.max_index(out=idxb[:, ds(t * 24, 8)], in_max=m8[:, 0:8],
                        in_values=a_ps)
    nc.vector.match_replace(out=a_ps, in_to_replace=m8[:, 0:8],
                            in_values=a_ps, imm_value=NEG_BIG)
    nc.vector.max(out=m8[:, 8:16], in_=a_ps)
    nc.vector.max_index(out=idxb[:, ds(t * 24 + 8, 8)], in_max=m8[:, 8:16],
                        in_values=a_ps)
    nc.vector.match_replace(out=a_ps, in_to_replace=m8[:, 8:16],
                            in_values=a_ps, imm_value=NEG_BIG)
    nc.vector.max(out=m8[:, 16:24], in_=a_ps)
    nc.vector.max_index(out=idxb[:, ds(t * 24 + 16, 8)], in_max=m8[:, 16:24],
                        in_values=a_ps)


def _emit_knn_tile(nc, kaug_b, kaug_a, ncon, stage, psum, m8v, idxb, t):
    """One kNN row-tile: stage lhsT slab, 4 matmuls, exact top-24."""
    nc.vector.tensor_copy(out=stage, in_=kaug_b[:, ds(t * 128, 128)])
    for c in range(4):
        sl = slice(c * 512, (c + 1) * 512)
        nc.tensor.matmul(psum[:, sl], lhsT=stage, rhs=kaug_a[:, sl],
                         start=True, stop=True)
    _emit_topk(nc, psum, m8v, idxb, t)


def build_core_program(skip=()):
    nc = bacc.Bacc("TRN2", target_bir_lowering=False, debug=False)

    xaug_d = nc.declare_dram_parameter("xaug", [G, 4, P], f32, isOutput=False)
    wd = {n: nc.declare_dram_parameter(n, s, f32, isOutput=False)
          for n, s in W_NAMES.items()}
    out_d = nc.declare_dram_parameter("out", [G, 128], f32, isOutput=True)
    idx_dram = nc.dram_tensor("idx_scratch", [G, 2, P, K], i16)

    with TileContext(nc) as tc:
        with tc.tile_pool(name="const", bufs=1) as const, \
             tc.tile_pool(name="persist", bufs=1) as persist, \
             tc.tile_pool(name="pbig", bufs=1, space="PSUM") as pbig_pool:

            w_sb = {}
            for n, s in W_NAMES.items():
                w_sb[n] = const.tile(s, f32, tag=f"w_{n}", name=f"w_{n}")
                nc.sync.dma_start(out=w_sb[n], in_=wd[n][:, :])
            onescol = const.tile([128, 1], f32, tag="onescol")
            nc.vector.memset(onescol, 1.0)
            ones_row = const.tile([1, P], f32, tag="ones_row")
            nc.vector.memset(ones_row, 1.0)

            # persistent workspaces, reused across graphs/phases
            kaug1_a = persist.tile([5, P], f32, tag="k1a", name="k1a")
            kaug1_b = persist.tile([5, P], f32, tag="k1b", name="k1b")
            kaug2_a = persist.tile([66, P], f32, tag="k2a", name="k2a")
            kaug2_b = persist.tile([66, P], f32, tag="k2b", name="k2b")
            scrA = persist.tile([64, P], f32, tag="scrA", name="scrA")
            stage1 = persist.tile([5, 128], f32, tag="st1", name="st1")
            stage1b = persist.tile([5, 128], f32, tag="st1b", name="st1b")
            stage2 = persist.tile([66, 128], f32, tag="st2", name="st2")
            stage2b = persist.tile([66, 128], f32, tag="st2b", name="st2b")
            m8 = persist.tile([128, 48], f32, tag="m8", name="m8")
            idxb1 = persist.tile([128, NT * 24], u16, tag="ixb1", name="ixb1")
            idxb2 = persist.tile([128, NT * 24], u16, tag="ixb2", name="ixb2")
            ubuf = persist.tile([128, P], f32, tag="ubuf", name="ubuf")
            xaug2 = persist.tile([8, P // 2], f32, tag="xaug2", name="xaug2")
            a2 = persist.tile([128, P // 2], f32, tag="a2", name="a2")
            gath = persist.tile([128, EH], f32, tag="gath", name="gath")
            x1_2s = persist.tile([128, P // 2], f32, tag="x12s", name="x12s")
            x2full = persist.tile([128, P], f32, tag="x2f", name="x2f")
            idxw1 = persist.tile([128, EH // 16], i16, tag="ixw1", name="ixw1")
            idxw2a = persist.tile([128, EH // 16], i16, tag="ixw2a",
                                  name="ixw2a")
            idxw2b = persist.tile([128, EH // 16], i16, tag="ixw2b",
                                  name="ixw2b")
            outacc = persist.tile([128, G], f32, tag="oacc", name="oacc")

            pbig = pbig_pool.tile([128, 4096], f32, tag="pbig", name="pbig")
            apbig = pbig[:, 0:2048]
            bpbig = pbig[:, 2048:4096]

            # hoisted constant rows of the kNN aug tensors (engine ops must
            # start at partition 0/32/64/96 — misaligned rows go via DMA)
            nc.sync.dma_start(out=kaug1_b[4:5], in_=ones_row)
            nc.vector.memset(kaug2_a[64:65], 1.0)
            nc.sync.dma_start(out=kaug2_b[65:66], in_=ones_row)
            if "topk" in skip:
                nc.vector.memset(idxb1, 0)
                nc.vector.memset(idxb2, 0)
            if "idxw" in skip:
                nc.vector.memset(idxw1, 0)
                nc.vector.memset(idxw2a, 0)
                nc.vector.memset(idxw2b, 0)

            with tc.For_i(0, G) as g:
                # ======== kNN1 aug: rhs=[x;1;-d2], lhsT=[2x;-d2;1] ========
                nc.sync.dma_start(out=kaug1_a[0:4],
                                  in_=xaug_d[ds(g, 1)].squeeze(0))
                xsqn = scrA[0:3]
                nc.vector.scalar_tensor_tensor(
                    out=xsqn, in0=kaug1_a[0:3], scalar=-1.0,
                    in1=kaug1_a[0:3],
                    op0=mybir.AluOpType.mult, op1=mybir.AluOpType.mult)
                for c in range(4):
                    sl = slice(c * 512, (c + 1) * 512)
                    nc.tensor.matmul(pbig[0:1, sl], lhsT=onescol[0:3],
                                     rhs=xsqn[:, sl], start=True, stop=True)
                nc.vector.tensor_copy(out=scrA[0:1], in_=pbig[0:1, 0:P])
                nc.sync.dma_start(out=kaug1_a[4:5], in_=scrA[0:1])
                nc.sync.dma_start(out=kaug1_b[3:4], in_=scrA[0:1])
                nc.vector.tensor_scalar(
                    out=kaug1_b[0:3], in0=kaug1_a[0:3], scalar1=2.0,
                    scalar2=None, op0=mybir.AluOpType.mult)

                with tc.For_i(0, NT, 2) as t:
                    _emit_knn_tile(nc, kaug1_b, kaug1_a, 5, stage1,
                                   apbig, m8[:, 0:24], idxb1, t)
                    _emit_knn_tile(nc, kaug1_b, kaug1_a, 5, stage1b,
                                   bpbig, m8[:, 24:48], idxb1, t + 1)
                nc.sync.dma_start(
                    out=idx_dram[ds(g, 1)].squeeze(0)[0].rearrange(
                        "(t pp) k -> pp t k", pp=128),
                    in_=idxb1.rearrange("pp (t k) -> pp t k", k=24)[:, :, 0:K]
                        .bitcast(i16))

                # ======== conv1 ========
                for c in range(4):
                    sl = slice(c * 512, (c + 1) * 512)
                    nc.tensor.matmul(apbig[:, sl], lhsT=w_sb["wu_rep"],
                                     rhs=kaug1_a[0:4, sl],
                                     start=True, stop=True)
                nc.vector.tensor_copy(out=ubuf, in_=apbig)
                nc.vector.tensor_copy(out=xaug2[0:4],
                                      in_=kaug1_a[0:4, 0:P // 2])
                nc.sync.dma_start(out=xaug2[4:8],
                                  in_=kaug1_a[0:4, P // 2:P])
                for c in range(2):
                    sl = slice(c * 512, (c + 1) * 512)
                    nc.tensor.matmul(pbig[:, sl], lhsT=w_sb["wa_blk"],
                                     rhs=xaug2[:, sl], start=True, stop=True)
                nc.vector.tensor_copy(out=a2, in_=pbig[:, 0:P // 2])

                flat1 = idx_dram[ds(g, 1)].squeeze(0)[0].rearrange(
                    "p k -> (p k)")
                if "idxw" not in skip:
                    for grp in range(8):
                        h = grp // 4
                        src = flat1[h * EH:(h + 1) * EH].rearrange(
                            "(w q) -> q w", q=16)
                        nc.sync.dma_start(
                            out=idxw1[grp * 16:(grp + 1) * 16, :], in_=src)
                if "gath1" not in skip:
                    nc.gpsimd.ap_gather(
                        out_ap=gath.rearrange("p (n d) -> p n d", d=1),
                        in_ap=ubuf.rearrange("p (n d) -> p n d", d=1),
                        idxs_ap=idxw1,
                        channels=128, num_elems=P, d=1, num_idxs=EH)
                nc.vector.scalar_tensor_tensor(
                    out=gath.rearrange("p (n k) -> p n k", k=K),
                    in0=gath.rearrange("p (n k) -> p n k", k=K),
                    scalar=0.0,
                    in1=a2.unsqueeze(2).to_broadcast([128, P // 2, K]),
                    op0=mybir.AluOpType.bypass, op1=mybir.AluOpType.add)
                nc.vector.tensor_scalar(
                    out=gath, in0=gath, scalar1=0.0, scalar2=None,
                    op0=mybir.AluOpType.max)
                with tc.For_i(0, 8, 2) as c:
                    for u in range(2):
                        for k in range(5):
                            nc.tensor.matmul(
                                pbig[:, k * 512:(k + 1) * 512],
                                lhsT=w_sb["w2_blk"],
                                rhs=gath[:, ds((c + u) * 2560 + k * 512,
                                               512)],
                                start=True, stop=True)
                        nc.vector.tensor_reduce(
                            out=x1_2s[:, ds((c + u) * 128, 128)],
                            in_=pbig[:, 0:2560].rearrange(
                                "p (n k) -> p n k", k=K),
                            axis=mybir.AxisListType.X,
                            op=mybir.AluOpType.max)
                nc.vector.tensor_scalar(
                    out=x1_2s, in0=x1_2s, scalar1=w_sb["b2_2"], scalar2=None,
                    op0=mybir.AluOpType.add)
                nc.vector.tensor_copy(out=kaug2_a[0:64, 0:P // 2],
                                      in_=x1_2s[0:64])
                nc.vector.tensor_copy(out=kaug2_a[0:64, P // 2:P],
                                      in_=x1_2s[64:128])

                # ======== kNN2 aug: rhs=[x1;1;-d2], lhsT=[2x1;-d2;1] ======
                nc.vector.scalar_tensor_tensor(
                    out=scrA, in0=kaug2_a[0:64], scalar=-1.0,
                    in1=kaug2_a[0:64],
                    op0=mybir.AluOpType.mult, op1=mybir.AluOpType.mult)
                for c in range(4):
                    sl = slice(c * 512, (c + 1) * 512)
                    nc.tensor.matmul(pbig[0:1, sl], lhsT=onescol[0:64],
                                     rhs=scrA[:, sl], start=True, stop=True)
                nc.vector.tensor_copy(out=ubuf[0:1], in_=pbig[0:1, 0:P])
                nc.sync.dma_start(out=kaug2_a[65:66], in_=ubuf[0:1])
                nc.vector.tensor_copy(out=kaug2_b[64:65], in_=ubuf[0:1])
                nc.vector.tensor_scalar(
                    out=kaug2_b[0:64], in0=kaug2_a[0:64], scalar1=2.0,
                    scalar2=None, op0=mybir.AluOpType.mult)

                with tc.For_i(0, NT, 2) as t:
                    _emit_knn_tile(nc, kaug2_b, kaug2_a, 66, stage2,
                                   apbig, m8[:, 0:24], idxb2, t)
                    _emit_knn_tile(nc, kaug2_b, kaug2_a, 66, stage2b,
                                   bpbig, m8[:, 24:48], idxb2, t + 1)
                nc.sync.dma_start(
                    out=idx_dram[ds(g, 1)].squeeze(0)[1].rearrange(
                        "(t pp) k -> pp t k", pp=128),
                    in_=idxb2.rearrange("pp (t k) -> pp t k", k=24)[:, :, 0:K]
                        .bitcast(i16))

                # ======== conv2: u per point, gather, segmented max ======
                for c in range(4):
                    sl = slice(c * 512, (c + 1) * 512)
                    nc.tensor.matmul(apbig[:, sl], lhsT=w_sb["w3b"],
                                     rhs=kaug2_a[0:64, sl],
                                     start=True, stop=True)
                nc.vector.tensor_copy(out=ubuf, in_=apbig)

                flat2 = idx_dram[ds(g, 1)].squeeze(0)[1].rearrange(
                    "p k -> (p k)")
                if "idxw" not in skip:
                    for h, idxw2 in ((0, idxw2a), (1, idxw2b)):
                        src = flat2[h * EH:(h + 1) * EH].rearrange(
                            "(w q) -> q w", q=16)
                        for grp in range(8):
                            nc.sync.dma_start(
                                out=idxw2[grp * 16:(grp + 1) * 16, :],
                                in_=src)
                if "gath2" not in skip:
                    for h, idxw2 in ((0, idxw2a), (1, idxw2b)):
                        nc.gpsimd.ap_gather(
                            out_ap=gath.rearrange("p (n d) -> p n d", d=1),
                            in_ap=ubuf.rearrange("p (n d) -> p n d", d=1),
                            idxs_ap=idxw2,
                            channels=128, num_elems=P, d=1, num_idxs=EH)
                        nc.vector.tensor_reduce(
                            out=x2full[:, h * (P // 2):(h + 1) * (P // 2)],
                            in_=gath.rearrange("p (n k) -> p n k", k=K),
                            axis=mybir.AxisListType.X,
                            op=mybir.AluOpType.max)
                # c-part: x1@(W3a-W3b)+b3 per point, into PSUM, add post-max
                for c in range(4):
                    sl = slice(c * 512, (c + 1) * 512)
                    nc.tensor.matmul(apbig[:, sl], lhsT=w_sb["w3c_aug"],
                                     rhs=kaug2_a[0:65, sl],
                                     start=True, stop=True)
                nc.vector.scalar_tensor_tensor(
                    out=x2full, in0=x2full, scalar=0.0, in1=apbig,
                    op0=mybir.AluOpType.bypass, op1=mybir.AluOpType.add)

                # ======== lin1 + global max ========
                for c in range(4):
                    sl = slice(c * 512, (c + 1) * 512)
                    nc.tensor.matmul(apbig[:, sl], lhsT=w_sb["wl1_aug"],
                                     rhs=kaug2_a[0:65, sl],
                                     start=True, stop=False)
                    nc.tensor.matmul(apbig[:, sl], lhsT=w_sb["wl2"],
                                     rhs=x2full[:, sl],
                                     start=False, stop=True)
                nc.vector.tensor_reduce(
                    out=outacc[:, ds(g, 1)], in_=apbig,
                    axis=mybir.AxisListType.X, op=mybir.AluOpType.max)

            nc.sync.dma_start(out=out_d.rearrange("g f -> f g"), in_=outacc)
    nc.compile()
    return nc


def _get_program():
    if "nc" not in _cache:
        _cache["nc"] = build_core_program()
    return _cache["nc"]


def kernel(pos, W1, b1, g1, bt1, W2, b2, W3, b3, Wl, bl):
    pos = np.asarray(pos, np.float32)
    host = _build_host_tensors(
        np.asarray(W1, np.float32), np.asarray(b1, np.float32),
        np.asarray(g1, np.float32), np.asarray(bt1, np.float32),
        np.asarray(W2, np.float32), np.asarray(b2, np.float32),
        np.asarray(W3, np.float32), np.asarray(b3, np.float32),
        np.asarray(Wl, np.float32), np.asarray(bl, np.float32))

    nc = _get_program()
    in_maps = []
    for cid in range(NCORES):
        xs = pos[cid * G:(cid + 1) * G]                       # [G,2048,3]
        xaug = np.concatenate(
            [np.transpose(xs, (0, 2, 1)),
             np.ones((G, 1, P), np.float32)], axis=1)         # [G,4,2048]
        m = dict(xaug=np.ascontiguousarray(xaug))
        for n in W_NAMES:
            m[n] = np.ascontiguousarray(host[n], np.float32)
        in_maps.append(m)

    res = run_bass_kernel_spmd(nc, in_maps, list(range(NCORES)))
    outs = [np.asarray(om["out"]) for om in res.results]
    return np.concatenate(outs, axis=0).astype(np.float32)
